# revision 29
# baseline (speedup 1.0000x reference)
"""Trainium2 Bass kernel for nn_Attention_3934190044008.

Multi-head attention with additive bias and sigmoid gating:
  q = (q_x @ w_q) / 8, k = kv_x @ w_k, v = kv_x @ w_v   (8 heads x 64)
  a = softmax(q k^T + bias);  o = a @ v
  o = o * sigmoid(q_x @ w_g + b_g);  out = o @ w_o + b_o

Sharding: 16 (batch, head) pairs over 8 cores -> each core owns one batch
element and 2 heads, produces per-head unnormalized partial outputs
(o*g)^T @ w_o plus the softmax denominators; the host divides by the
denominators (1/rs commutes through the linear w_o), sums the partials
per batch, and adds b_o.

Key layout/engine choices (v4):
- Scores computed transposed, S^T [k, q]; softmax-over-k needs no
  partition reduction; denominator rides the AV matmul via a ones column
  in V.
- Two bias paths, chosen per k-tile (INJECT_EVERY knob):
  * mult path: host ships exp(bias-3) fp16; ACT does E0=exp(s-3) straight
    from PSUM; DVE multiplies E=E0*eb in 16-bit 2x mode.
  * inject path: host ships raw bias fp16; the PE adds it into the score
    PSUM via an identity-stationary matmul and ACT does E=exp(s+b-6)
    directly.  This doubles as PE "filler": the HAM clock gate only stays
    at 2.4 GHz while the PE is nearly gap-free, so the PE should be the
    (slightly) slowest engine in the attention loop.
- AV matmuls run one k-tile behind QK (software pipelining) so the PE
  never waits on the exp/mult chain of the current tile.
- All matmuls fp16 at 1 cyc/row (issue rate ~216ns per 512-row MM warm).
  Gate via tanh (same ACT table set as exp -> one table load):
  sigmoid(z) = 0.5 (1 + tanh(z/2)), 0.5 folded into w_o, (1 + t) fused
  via scalar_tensor_tensor; t row 64 = 0 lets the denominator ride
  through the same op.
- Head 0's output projection + output DMA retire inside head 1's
  attention loop; only head 1's runs in the tail.
"""

import os
import sys
import threading
from contextlib import ExitStack

import numpy as np

_REPO = "/opt/trn_rl_repo"
if _REPO not in sys.path and os.path.isdir(_REPO):
    sys.path.insert(0, _REPO)

import concourse.bass as bass  # noqa: E402
import concourse.mybir as mybir  # noqa: E402
import concourse.tile as tile  # noqa: E402
from concourse import bacc  # noqa: E402
from concourse.bass_utils import run_bass_kernel_spmd  # noqa: E402

F32 = mybir.dt.float32
F16 = mybir.dt.float16

B, SEQ, CQ = 2, 2048, 256
H, DH = 8, 64
HD = H * DH  # 512
N_CORES = 8
HPC = 2  # heads per core
NKT = SEQ // 128  # 16 k-tiles
P = 128
QB = 1024  # q block (one S/E tile)
NQB = SEQ // QB
SHIFT = 3.0  # total softmax shift is 2*SHIFT; cancels in the division

# kt % INJECT_EVERY == INJECT_EVERY-1 -> PE-inject bias path (0 = never)
INJECT_EVERY = int(os.environ.get("KRN_INJECT_EVERY", "4"))


def _is_inject(kt):
    return INJECT_EVERY > 0 and kt % INJECT_EVERY == INJECT_EVERY - 1


def build_nc():
    nc = bacc.Bacc("TRN2", target_bir_lowering=False, debug=False)

    # both inputs in one tensor: single DMA issue (SP issues cost ~650ns each)
    xT = nc.dram_tensor("xT", [2, CQ, SEQ], F16, kind="ExternalInput").ap()
    wkq = nc.dram_tensor("wkq", [2, CQ, HPC * DH], F16, kind="ExternalInput").ap()
    ebT = nc.dram_tensor("ebT", [HPC, SEQ, SEQ], F16, kind="ExternalInput").ap()
    wv = nc.dram_tensor("wv", [CQ, HPC * DH], F16, kind="ExternalInput").ap()
    wg = nc.dram_tensor("wg", [CQ, HPC * DH], F16, kind="ExternalInput").ap()
    bgh = nc.dram_tensor("bgh", [DH, HPC], F32, kind="ExternalInput").ap()
    wo = nc.dram_tensor("wo", [HPC * DH, CQ], F16, kind="ExternalInput").ap()
    ident = nc.dram_tensor("ident", [P, P], F16, kind="ExternalInput").ap()
    out0_d = nc.dram_tensor("out0", [SEQ, CQ], F16, kind="ExternalOutput").ap()
    out1_d = nc.dram_tensor("out1", [SEQ, CQ], F16, kind="ExternalOutput").ap()
    rs_d = nc.dram_tensor("rs", [1, HPC, SEQ], F16, kind="ExternalOutput").ap()

    with tile.TileContext(nc) as tc:
        with ExitStack() as ctx:
            singles = ctx.enter_context(tc.tile_pool(name="singles", bufs=1))
            biaspool = ctx.enter_context(tc.tile_pool(name="biasp", bufs=6))

            # K/Q weights first on the fast SP ring (they gate the first
            # projection matmuls), then the inputs; everything else rides
            # the gpsimd SWDGE queue.
            w_sbs = {}
            wkq_sb = singles.tile([P, 2, 2, P], F16)  # [p, k|q, a, cols]
            nc.sync.dma_start(
                wkq_sb, wkq.rearrange("w (a p) c -> p w a c", p=P))
            w_sbs["wk"] = wkq_sb[:, 0, :, :]
            w_sbs["wq"] = wkq_sb[:, 1, :, :]
            xT_sb = singles.tile([P, 2, 2, SEQ], F16)  # [p, kv|q, a, seq]
            for w in range(2):  # kv first: it gates the first matmuls
                nc.sync.dma_start(xT_sb[:, w, :, :],
                                  xT[w].rearrange("(a p) s -> p a s", p=P))
            kvxT_sb = xT_sb[:, 0, :, :]
            qxT_sb = xT_sb[:, 1, :, :]
            for name, src in (("wv", wv), ("wg", wg)):
                t = singles.tile([P, 2, P], F16, tag=f"w_{name}")
                nc.gpsimd.dma_start(t, src.rearrange("(a p) c -> p a c", p=P))
                w_sbs[name] = t
            wo_sb = singles.tile([DH, HPC, CQ], F16)
            nc.gpsimd.dma_start(wo_sb, wo.rearrange("(h p) c -> p h c", p=DH))
            bgh_sb = singles.tile([DH, HPC], F32)
            nc.gpsimd.dma_start(bgh_sb, bgh)
            I_sb = singles.tile([P, P], F16)
            nc.gpsimd.dma_start(I_sb, ident)

            KT_sb = singles.tile([P, SEQ], F16)   # [2h x 64 d, k]
            QT_sb = singles.tile([P, SEQ], F16)   # [2h x 64 d, q]
            t_sb = singles.tile([DH + 1, HPC, SEQ], F16)  # tanh gate; row 64 = 0
            V_sb = singles.tile([P, HPC, NKT, DH + 1], F16)  # [k%128, h, kt, d|1]
            OG_sb = singles.tile([DH + 1, HPC, SEQ], F16)  # (1+t)*OT; row 64 = rs
            out_sb = singles.tile([P, NKT, HPC, CQ], F16)
            shift3_sb = singles.tile([P, 1], F32)
            shift6_sb = singles.tile([P, 1], F32)
            dummy_sb = singles.tile([P, 512], F16)
            warm_act = singles.tile([P, 1], F32)
            nc.vector.memset(dummy_sb, 0.0)
            nc.vector.memset(V_sb[:, :, :, DH:DH + 1], 1.0)
            nc.vector.memset(t_sb[DH:DH + 1, :, :], 0.0)
            nc.vector.memset(shift3_sb, -SHIFT)
            nc.vector.memset(shift6_sb, -2.0 * SHIFT)

            # pre-issue the first bias tiles so the attention fill phase
            # never stalls on DMA
            eb_tiles = {}
            for kt in range(5):
                eb = biaspool.tile([P, SEQ], F16)
                nc.gpsimd.dma_start(eb, ebT[0, bass.ts(kt, P), :])
                eb_tiles[(0, kt)] = eb

            # ---- stage B: projections as one dense back-to-back MM block
            # (HAM warmup burst).  ACT fills its idle prologue with the
            # gate tanh chunks.
            with tc.tile_pool(name="ppsum", bufs=2, space="PSUM") as ppool:
                # V projection; out rows = tokens(k), cols = 2 heads x 64.
                # First half runs in the prologue (kv-gated work that covers
                # the qx DMA wait), the rest as head-0 attention PE filler.
                def v_proj(kt, pool, tag):
                    ps = pool.tile([P, P], F32, tag=tag, name="vp_ps")
                    nc.tensor.matmul(ps, kvxT_sb[:, 0, bass.ts(kt, P)],
                                     w_sbs["wv"][:, 0, :], start=True, stop=False)
                    nc.tensor.matmul(ps, kvxT_sb[:, 1, bass.ts(kt, P)],
                                     w_sbs["wv"][:, 1, :], start=False, stop=True)
                    nc.vector.tensor_copy(
                        V_sb[:, :, kt, 0:DH],
                        ps.rearrange("p (h d) -> p h d", h=HPC))

                # pre-warm: dummy back-to-back matmuls with no DMA deps run
                # while the inputs stream in, releasing the HAM clock gate
                # (K=8/8) before the real work starts
                # also pre-load the ACT exp/tanh table set while DMAs stream
                nc.scalar.activation(warm_act, shift3_sb,
                                     mybir.ActivationFunctionType.Exp)
                for _ in range(24):
                    ps = ppool.tile([P, 512], F32, tag="warm")
                    nc.tensor.matmul(ps, dummy_sb[:, 0:P], dummy_sb,
                                     start=True, stop=True)
                for wt, x_sb, dst in ((w_sbs["wk"], kvxT_sb, KT_sb),
                                      (w_sbs["wq"], qxT_sb, QT_sb)):
                    for tt in range(SEQ // 512):
                        ps = ppool.tile([P, 512], F32, tag="proj")
                        nc.tensor.matmul(ps, wt[:, 0, :],
                                         x_sb[:, 0, bass.ts(tt, 512)],
                                         start=True, stop=False)
                        nc.tensor.matmul(ps, wt[:, 1, :],
                                         x_sb[:, 1, bass.ts(tt, 512)],
                                         start=False, stop=True)
                        nc.vector.tensor_copy(dst[:, bass.ts(tt, 512)], ps)
                        if wt is w_sbs["wk"]:
                            # kv-only work right after the KT chunks so the
                            # PE never idles waiting for the qx DMA
                            v_proj(2 * tt, ppool, "vproj")
                            v_proj(2 * tt + 1, ppool, "vproj")
                # gate per head (features on partitions 0-63 to match OT)
                for h in range(HPC):
                    hc = slice(h * DH, (h + 1) * DH)
                    for tt in range(SEQ // 512):
                        ps = ppool.tile([DH, 512], F32, tag="gproj")
                        nc.tensor.matmul(ps, w_sbs["wg"][:, 0, hc],
                                         qxT_sb[:, 0, bass.ts(tt, 512)],
                                         start=True, stop=False)
                        nc.tensor.matmul(ps, w_sbs["wg"][:, 1, hc],
                                         qxT_sb[:, 1, bass.ts(tt, 512)],
                                         start=False, stop=True)
                        nc.scalar.activation(t_sb[0:DH, h, bass.ts(tt, 512)], ps,
                                             mybir.ActivationFunctionType.Tanh,
                                             bias=bgh_sb[:, h:h + 1], scale=0.5)

            # ---- stage C: attention ----
            with tc.tile_pool(name="spsum", bufs=2, space="PSUM") as spool, \
                 tc.tile_pool(name="otpsum", bufs=2, space="PSUM") as otpool, \
                 tc.tile_pool(name="e0p", bufs=6) as e0pool:

                def out_proj(tt, h):
                    # unnormalized per-head partial: (OG_h)^T @ (0.5 wo_h);
                    # rides spool's slots (no separate PSUM budget)
                    ps = spool.tile([P, CQ], F32, tag="s", name="fin_ps")
                    nc.tensor.matmul(ps, OG_sb[0:DH, h, bass.ts(tt, P)],
                                     wo_sb[:, h, :], start=True, stop=True)
                    nc.vector.tensor_copy(out_sb[:, tt, h, :], ps)

                def out_dma(tt_hi, h):
                    # DMA 4 token-tiles of head h once their copies landed
                    dst = (out0_d, out1_d)[h].rearrange("(t p) c -> p t c", p=P)
                    nc.sync.dma_start(dst[:, tt_hi - 3:tt_hi + 1, :],
                                      out_sb[:, tt_hi - 3:tt_hi + 1, h, :])

                for h in range(HPC):
                    hsl = slice(h * DH, (h + 1) * DH)
                    OTs = [otpool.tile([DH + 1, QB], F32, name=f"OT{h}_{qb}",
                                       tag="ot")
                           for qb in range(NQB)]
                    Es_prev = None

                    def av(kt, Es):
                        for qb in range(NQB):
                            for j in range(2):
                                nc.tensor.matmul(
                                    OTs[qb][:, bass.ts(j, 512)],
                                    V_sb[:, h, kt, :],
                                    Es[qb][:, bass.ts(j, 512)],
                                    start=(kt == 0), stop=(kt == NKT - 1))

                    for kt in range(NKT):
                        inj = _is_inject(kt)
                        if (h, kt) in eb_tiles:
                            eb_sb = eb_tiles.pop((h, kt))
                        else:
                            eb_sb = biaspool.tile([P, SEQ], F16)
                            nc.sync.dma_start(eb_sb, ebT[h, bass.ts(kt, P), :])
                        Ss, Es = [], []
                        for qb in range(NQB):
                            S = spool.tile([P, QB], F32, tag="s")
                            for j in range(2):
                                nc.tensor.matmul(
                                    S[:, bass.ts(j, 512)],
                                    KT_sb[hsl, bass.ts(kt, P)],
                                    QT_sb[hsl, bass.ds(qb * QB + j * 512, 512)],
                                    start=True, stop=not inj)
                            Ss.append(S)
                        if inj:
                            # PE adds the bias into the score PSUM
                            for qb in range(NQB):
                                for j in range(2):
                                    nc.tensor.matmul(
                                        Ss[qb][:, bass.ts(j, 512)],
                                        I_sb,
                                        eb_sb[:, bass.ds(qb * QB + j * 512, 512)],
                                        start=False, stop=True)
                        # PE filler between QK and AV bursts: head 0 projects
                        # the remaining V tiles, head 1 retires head 0's
                        # out-proj + DMA (kt-1 lag: the h0 epilogue is done)
                        if h == 0 and 2 <= kt <= 5:
                            v_proj(2 * kt + 4, spool, "s")
                            v_proj(2 * kt + 5, spool, "s")
                        elif h == 1 and kt >= 1:
                            out_proj(kt - 1, 0)
                            if kt % 4 == 0:
                                out_dma(kt - 1, 0)
                        # AV for the previous k-tile (its E chain is done)
                        if Es_prev is not None:
                            av(kt - 1, Es_prev)
                        for qb in range(NQB):
                            E = e0pool.tile([P, QB], F16, tag="e0")
                            nc.scalar.activation(
                                E, Ss[qb], mybir.ActivationFunctionType.Exp,
                                bias=shift6_sb if inj else shift3_sb)
                            if not inj:
                                # in-place: all-fp16 SBUF -> DVE 2x mode
                                nc.vector.tensor_mul(
                                    E, E, eb_sb[:, bass.ds(qb * QB, QB)])
                            Es.append(E)
                        Es_prev = Es
                    av(NKT - 1, Es_prev)
                    # epilogue: OG = (1 + t) * OT; row 64 (t=0) passes rs
                    for qb in range(NQB):
                        qsl = bass.ds(qb * QB, QB)
                        nc.vector.scalar_tensor_tensor(
                            OG_sb[:, h, qsl], t_sb[:, h, qsl], 1.0, OTs[qb],
                            op0=mybir.AluOpType.add, op1=mybir.AluOpType.mult)
                    if h == HPC - 1:
                        nc.sync.dma_start(rs_d, OG_sb[DH:DH + 1, :, :])

                # ---- stage D: tail (out-proj copies split DVE/ACT so the
                # two engines drain the 16 tiles in parallel) ----
                out_proj(NKT - 1, 0)
                out_dma(NKT - 1, 0)
                for tt in range(SEQ // P):
                    ps = spool.tile([P, CQ], F32, tag="s", name="fin_ps")
                    nc.tensor.matmul(ps, OG_sb[0:DH, 1, bass.ts(tt, P)],
                                     wo_sb[:, 1, :], start=True, stop=True)
                    if tt % 2 == 0:
                        nc.vector.tensor_copy(out_sb[:, tt, 1, :], ps)
                    else:
                        nc.scalar.copy(out_sb[:, tt, 1, :], ps)
                    if tt % 4 == 3 and tt < 12:
                        out_dma(tt, 1)
                for tt in (13, 15):
                    nc.sync.dma_start(
                        out1_d.rearrange("(t p) c -> p t c", p=P)[
                            :, tt - 1:tt + 1, :],
                        out_sb[:, tt - 1:tt + 1, 1, :])

    nc.compile()
    return nc


_NC = None
_NC_LOCK = threading.Lock()


def _get_nc():
    global _NC
    with _NC_LOCK:
        if _NC is None:
            _NC = build_nc()
        return _NC


def make_in_maps(q_x, kv_x, bias, w_q, w_k, w_v, w_g, b_g, w_o, b_o):
    del b_o  # added on the host after the gather
    q_x = np.asarray(q_x, dtype=np.float32)
    kv_x = np.asarray(kv_x, dtype=np.float32)
    bias = np.asarray(bias, dtype=np.float32)
    w_q = np.asarray(w_q, dtype=np.float32) * np.float32(0.125)  # fold 1/sqrt(64)
    w_k = np.asarray(w_k, dtype=np.float32)
    w_v = np.asarray(w_v, dtype=np.float32)
    w_g = np.asarray(w_g, dtype=np.float32)
    b_g = np.asarray(b_g, dtype=np.float32)
    w_o = np.asarray(w_o, dtype=np.float32) * np.float32(0.5)  # tanh gate trick

    qxT = {}
    kvxT = {}
    for b in range(B):
        qxT[b] = np.ascontiguousarray(q_x[b].T.astype(np.float16))
        kvxT[b] = np.ascontiguousarray(kv_x[b].T.astype(np.float16))
    ident = np.eye(P, dtype=np.float16)

    in_maps = []
    for c in range(N_CORES):
        b = c // (N_CORES // B)
        h0 = HPC * (c % (N_CORES // B))
        cols = slice(h0 * DH, (h0 + HPC) * DH)
        # per-kt rows: raw bias for inject k-tiles, exp(bias-3) for mult
        bT = np.ascontiguousarray(bias[b, h0:h0 + HPC].swapaxes(1, 2))
        ebT = np.empty((HPC, SEQ, SEQ), dtype=np.float16)
        for kt in range(NKT):
            rows = slice(kt * P, (kt + 1) * P)
            if _is_inject(kt):
                ebT[:, rows, :] = bT[:, rows, :].astype(np.float16)
            else:
                ebT[:, rows, :] = np.exp(
                    bT[:, rows, :] - np.float32(SHIFT)).astype(np.float16)
        in_maps.append({
            "xT": np.ascontiguousarray(np.stack([kvxT[b], qxT[b]])),
            "wkq": np.ascontiguousarray(np.stack([
                w_k[:, cols].astype(np.float16),
                w_q[:, cols].astype(np.float16)])),
            "ebT": ebT,
            "wv": np.ascontiguousarray(w_v[:, cols].astype(np.float16)),
            "wg": np.ascontiguousarray(w_g[:, cols].astype(np.float16)),
            "bgh": np.ascontiguousarray(
                (0.5 * b_g[cols]).reshape(HPC, DH).T.astype(np.float32)),
            "wo": np.ascontiguousarray(w_o[cols, :].astype(np.float16)),
            "ident": ident,
        })
    return in_maps


def gather_output(results, b_o):
    full = np.zeros((B, SEQ, CQ), dtype=np.float32)
    for c in range(N_CORES):
        b = c // (N_CORES // B)
        rs = results[c]["rs"][0].astype(np.float32)
        for h in range(HPC):
            out = results[c][f"out{h}"].astype(np.float32)
            full[b] += out / rs[h][:, None]
    full += np.asarray(b_o, dtype=np.float32)
    return full


def kernel(**inputs):
    nc = _get_nc()
    in_maps = make_in_maps(**inputs)
    res = run_bass_kernel_spmd(nc, in_maps, core_ids=list(range(N_CORES)))
    return gather_output(res.results, inputs["b_o"])
